# revision 18
# baseline (speedup 1.0000x reference)
"""BKT forward kernel for Trainium2 (8 NeuronCores, data-parallel over batch).

Math: in odds space rho = L/(1-L) the BKT update is affine:
    rho' = a_t * rho + lam,   a_t = y ? (1-s)/(g(1-l)) : s/((1-g)(1-l)),
and the clip L <= 1-EPS becomes rho <= R. Pin steps (clip binding) are
detected with a linear-space scaled scan W' = min(a*W, 1) (W = rho_mult/R,
fp32 scan state so no underflow for any reachable trajectory), threshold
W >= (R-lam)/R. The trajectory is then rebuilt with a mult/add scan whose
operands force state = R at pins.

Engine schedule (per 128-student tile, all [128,512]):
  Act : a    = y*d + a0                      -> fp16  (612ns)
  DVE : W    = scan(a*state min 1)           -> fp16  (594ns, fp32 state)
  Act : notm = sigmoid(W*(-K) + K*theta)     -> fp16  (612ns, saturated step)
  Pool: t1   = (W >= theta)*R                -> bf16  (~427ns)
  Pool: d1   = max(t1, lam)                  -> bf16  (~427ns)
  Pool: d0   = notm * a                      -> fp16  (~427ns)
  DVE : p    = scan(d0*state add d1)         -> bf16  (594ns, fp32 state)
  SP  : DMA y in, DMA p out (bf16)
Host computes lat = p/(1+p), cor = g+(1-s-g)*lat (bounded maps of p, so
bf16 output error stays ~0.4%).
"""

import numpy as np

B_FULL = 65536
T = 512
N_CORES = 8
B_CORE = B_FULL // N_CORES          # 8192
N_TILES = B_CORE // 128             # 64
EPS = 1e-6
NPAR = 8

_cache = {}


def _consts():
    f32 = np.float32
    Lstar = f32(1.0) - f32(EPS)
    R = f32(np.float64(Lstar) / (1.0 - np.float64(Lstar)))
    return float(R)


def _build_bass():
    import concourse.bacc as bacc
    import concourse.mybir as mybir
    from concourse.tile import TileContext

    R = _consts()
    dt = mybir.dt
    op = mybir.AluOpType
    act = mybir.ActivationFunctionType

    nc = bacc.Bacc(None, target_bir_lowering=False)
    y_d = nc.dram_tensor("y", [B_CORE, T], dt.int8, kind="ExternalInput")
    par_d = nc.dram_tensor("par", [128, N_TILES * NPAR], dt.float32, kind="ExternalInput")
    p_d = nc.dram_tensor("p", [B_CORE, T], dt.bfloat16, kind="ExternalOutput")

    # par slots: 0=a0, 1=d, 2=negKp, 3=Kb, 4=lam, 5=rho0, 6=theta, 7=w0
    with TileContext(nc) as tc:
        with (
            tc.tile_pool(name="const", bufs=1) as cpool,
            tc.tile_pool(name="work", bufs=8) as pool,
        ):
            ones16 = cpool.tile([128, T], dt.float16)
            nc.gpsimd.memset(ones16[:], 1.0)
            par_t = cpool.tile([128, N_TILES * NPAR], dt.float32)
            nc.sync.dma_start(par_t[:], par_d[:, :])
            # all engines read the DMA'd par tile directly
            par_gp = par_t
            par_dv = par_t
            par_ac = par_t

            # 2-stage software pipeline: stage A (dma/a/W) of tile j is
            # emitted before stage B (notm/d1/d0/p/dma) of tile j-1 so the
            # DVE runs W(j) while Act/Pool produce tile j-1's scan operands.
            stash = {}
            for j in range(N_TILES + 1):
                if j < N_TILES:
                    b = j * NPAR
                    y_t = pool.tile([128, T], dt.int8, tag="y")
                    nc.sync.dma_start(y_t[:], y_d[j * 128 : (j + 1) * 128, :])

                    a_t = pool.tile([128, T], dt.float16, tag="a")
                    if j % 7 < 4:
                        nc.scalar.activation(
                            a_t[:], y_t[:], act.Identity,
                            bias=par_ac[:, b + 0 : b + 1],
                            scale=par_ac[:, b + 1 : b + 2],
                        )
                    else:
                        nc.gpsimd.tensor_scalar(
                            a_t[:], y_t[:], par_gp[:, b + 1 : b + 2],
                            par_gp[:, b + 0 : b + 1], op.mult, op.add,
                        )

                    w_t = pool.tile([128, T], dt.float16, tag="w")
                    nc.vector.tensor_tensor_scan(
                        w_t[:], a_t[:], ones16[:], par_dv[:, b + 7 : b + 8],
                        op.mult, op.min,
                    )
                    stash[j] = (a_t, w_t)

                if j >= 1:
                    i = j - 1
                    b = i * NPAR
                    a_t, w_t = stash.pop(i)
                    # fp16 W is quantized: no values in (1-4.88e-4, 1), so a
                    # global threshold 0.99975 separates pinned (W==1) from
                    # unpinned. sigmoid arg = -140000*W + 139965: exactly -35
                    # at W=1 (notm->0), +33.4 at the next fp16 value below 1.
                    notm_t = pool.tile([128, T], dt.float16, tag="notm")
                    if i >= N_TILES - 2:
                        # drain tiles: keep the chain on Pool (skips an Act hop)
                        nc.gpsimd.tensor_scalar(
                            notm_t[:], w_t[:], 0.99975, None, op.is_lt
                        )
                    else:
                        nc.scalar.activation(
                            notm_t[:], w_t[:], act.Sigmoid,
                            bias=par_ac[:, b + 3 : b + 4], scale=-140000.0,
                        )

                    # R-scaled units: pin value is exactly 1 = is_ge output, so
                    # d1~ = max((W>=thr), lam/R) is a single fused op.
                    d1_t = pool.tile([128, T], dt.bfloat16, tag="d1")
                    nc.gpsimd.tensor_scalar(
                        d1_t[:], w_t[:], 0.99975, par_gp[:, b + 4 : b + 5],
                        op.is_ge, op.max,
                    )
                    d0_t = pool.tile([128, T], dt.float16, tag="d0")
                    nc.gpsimd.tensor_tensor(d0_t[:], notm_t[:], a_t[:], op.mult)

                    p_t = pool.tile([128, T], dt.bfloat16, tag="p")
                    nc.vector.tensor_tensor_scan(
                        p_t[:], d0_t[:], d1_t[:], par_dv[:, b + 7 : b + 8],
                        op.mult, op.add,
                    )

                    nc.sync.dma_start(p_d[i * 128 : (i + 1) * 128, :], p_t[:])
    nc.compile()
    return nc


def _host_params(X, learn_w, guess_w, slip_w, prior_w):
    f32 = np.float32
    f64 = np.float64

    def sig(w):
        return 1.0 / (1.0 + np.exp(-w.astype(f64)))

    l = sig(learn_w[X[:, 0], 0])
    g = sig(guess_w[X[:, 1], 0])
    s = sig(slip_w[X[:, 2], 0])
    p = sig(prior_w[X[:, 3], 0])
    R = f64(_consts())
    a1 = (1 - s) / (g * (1 - l))
    a0 = s / ((1 - g) * (1 - l))
    lam = l / (1 - l)
    rho0 = p / (1 - p)
    d = (a1 - a0).astype(f32)
    w0 = (rho0 / R).astype(f32)
    zero = np.zeros_like(d)
    kb = np.full_like(d, 139965.0)        # sigmoid bias const (see _build_bass)
    lamR = (lam / R).astype(f32)
    par = np.stack(
        [a0.astype(f32), d, zero, kb, lamR,
         rho0.astype(f32), zero, w0], axis=1,
    )
    par = par.reshape(N_CORES, N_TILES, 128, NPAR).transpose(0, 2, 1, 3)
    par = np.ascontiguousarray(par.reshape(N_CORES, 128, N_TILES * NPAR), dtype=f32)
    gk = g.astype(f32)
    ck = (1 - s - g).astype(f32)
    return par, gk, ck, p.astype(f32)


def kernel(X, y, learn_w, guess_w, slip_w, prior_w, _trace=False):
    from concourse import bass_utils

    X = np.asarray(X)
    y8 = np.ascontiguousarray(np.asarray(y, dtype=np.int8))
    par, gk, ck, p0 = _host_params(
        X,
        np.asarray(learn_w, np.float32),
        np.asarray(guess_w, np.float32),
        np.asarray(slip_w, np.float32),
        np.asarray(prior_w, np.float32),
    )

    if "nc" not in _cache:
        _cache["nc"] = _build_bass()
    nc = _cache["nc"]

    in_maps = [
        {"y": y8[i * B_CORE : (i + 1) * B_CORE], "par": par[i]}
        for i in range(N_CORES)
    ]
    res = bass_utils.run_bass_kernel_spmd(
        nc, in_maps, core_ids=list(range(N_CORES)), trace=_trace
    )
    outs = res.results
    p_all = np.concatenate(
        [np.asarray(outs[i]["p"]).astype(np.float32) for i in range(N_CORES)], axis=0
    )
    # p_all[:, t] = odds/R AFTER step t; latents are recorded BEFORE the update
    lat = np.empty((B_FULL, T), np.float32)
    lat[:, 0] = p0
    ptrim = p_all[:, : T - 1] * np.float32(_consts())
    lat[:, 1:] = ptrim / (1.0 + ptrim)
    cor = gk[:, None] + ck[:, None] * lat
    if _trace:
        _cache["last_exec_time_ns"] = res.exec_time_ns
    return cor, lat


# revision 19
# speedup vs baseline: 1.0283x; 1.0283x over previous
"""BKT forward kernel for Trainium2 (8 NeuronCores, data-parallel over batch).

Math: in odds space rho = L/(1-L) the BKT update is affine:
    rho' = a_t * rho + lam,   a_t = y ? (1-s)/(g(1-l)) : s/((1-g)(1-l)),
and the clip L <= 1-EPS becomes rho <= R. Pin steps (clip binding) are
detected with a linear-space scaled scan W' = min(a*W, 1) (W = rho_mult/R,
fp32 scan state so no underflow for any reachable trajectory), threshold
W >= (R-lam)/R. The trajectory is then rebuilt with a mult/add scan whose
operands force state = R at pins.

Engine schedule (per 128-student tile, all [128,512]):
  Act : a    = y*d + a0                      -> fp16  (612ns)
  DVE : W    = scan(a*state min 1)           -> fp16  (594ns, fp32 state)
  Act : notm = sigmoid(W*(-K) + K*theta)     -> fp16  (612ns, saturated step)
  Pool: t1   = (W >= theta)*R                -> bf16  (~427ns)
  Pool: d1   = max(t1, lam)                  -> bf16  (~427ns)
  Pool: d0   = notm * a                      -> fp16  (~427ns)
  DVE : p    = scan(d0*state add d1)         -> bf16  (594ns, fp32 state)
  SP  : DMA y in, DMA p out (bf16)
Host computes lat = p/(1+p), cor = g+(1-s-g)*lat (bounded maps of p, so
bf16 output error stays ~0.4%).
"""

import numpy as np

B_FULL = 65536
T = 512
N_CORES = 8
B_CORE = B_FULL // N_CORES          # 8192
N_TILES = B_CORE // 128             # 64
EPS = 1e-6
NPAR = 8

_cache = {}


def _consts():
    f32 = np.float32
    Lstar = f32(1.0) - f32(EPS)
    R = f32(np.float64(Lstar) / (1.0 - np.float64(Lstar)))
    return float(R)


def _build_bass():
    import concourse.bacc as bacc
    import concourse.mybir as mybir
    from concourse.tile import TileContext

    R = _consts()
    dt = mybir.dt
    op = mybir.AluOpType
    act = mybir.ActivationFunctionType

    nc = bacc.Bacc(None, target_bir_lowering=False)
    y_d = nc.dram_tensor("y", [B_CORE, T], dt.int8, kind="ExternalInput")
    par_d = nc.dram_tensor("par", [128, N_TILES * NPAR], dt.float32, kind="ExternalInput")
    p_d = nc.dram_tensor("p", [B_CORE, T], dt.bfloat16, kind="ExternalOutput")

    # par slots: 0=a0, 1=d, 2=negKp, 3=Kb, 4=lam, 5=rho0, 6=theta, 7=w0
    with TileContext(nc) as tc:
        with (
            tc.tile_pool(name="const", bufs=1) as cpool,
            tc.tile_pool(name="work", bufs=8) as pool,
        ):
            ones16 = cpool.tile([128, T], dt.float16)
            nc.gpsimd.memset(ones16[:], 1.0)
            par_t = cpool.tile([128, N_TILES * NPAR], dt.float32)
            nc.sync.dma_start(par_t[:], par_d[:, :])
            # per-engine copies so scalar-AP reads are same-engine deps
            par_gp = cpool.tile([128, N_TILES * NPAR], dt.float32)
            nc.gpsimd.tensor_copy(par_gp[:], par_t[:])
            par_dv = cpool.tile([128, N_TILES * NPAR], dt.float32)
            nc.vector.tensor_copy(par_dv[:], par_t[:])
            par_ac = cpool.tile([128, N_TILES * NPAR], dt.float32)
            nc.scalar.copy(par_ac[:], par_t[:])

            # 2-stage software pipeline: stage A (dma/a/W) of tile j is
            # emitted before stage B (notm/d1/d0/p/dma) of tile j-1 so the
            # DVE runs W(j) while Act/Pool produce tile j-1's scan operands.
            stash = {}
            for j in range(N_TILES + 1):
                if j < N_TILES:
                    b = j * NPAR
                    y_t = pool.tile([128, T], dt.int8, tag="y")
                    nc.sync.dma_start(y_t[:], y_d[j * 128 : (j + 1) * 128, :])

                    a_t = pool.tile([128, T], dt.float16, tag="a")
                    if j % 7 < 4:
                        nc.scalar.activation(
                            a_t[:], y_t[:], act.Identity,
                            bias=par_ac[:, b + 0 : b + 1],
                            scale=par_ac[:, b + 1 : b + 2],
                        )
                    else:
                        nc.gpsimd.tensor_scalar(
                            a_t[:], y_t[:], par_gp[:, b + 1 : b + 2],
                            par_gp[:, b + 0 : b + 1], op.mult, op.add,
                        )

                    w_t = pool.tile([128, T], dt.float16, tag="w")
                    nc.vector.tensor_tensor_scan(
                        w_t[:], a_t[:], ones16[:], par_dv[:, b + 7 : b + 8],
                        op.mult, op.min,
                    )
                    stash[j] = (a_t, w_t)

                if j >= 1:
                    i = j - 1
                    b = i * NPAR
                    a_t, w_t = stash.pop(i)
                    # fp16 W is quantized: no values in (1-4.88e-4, 1), so a
                    # global threshold 0.99975 separates pinned (W==1) from
                    # unpinned. sigmoid arg = -140000*W + 139965: exactly -35
                    # at W=1 (notm->0), +33.4 at the next fp16 value below 1.
                    notm_t = pool.tile([128, T], dt.float16, tag="notm")
                    if i >= N_TILES - 2:
                        # drain tiles: keep the chain on Pool (skips an Act hop)
                        nc.gpsimd.tensor_scalar(
                            notm_t[:], w_t[:], 0.99975, None, op.is_lt
                        )
                    else:
                        nc.scalar.activation(
                            notm_t[:], w_t[:], act.Sigmoid,
                            bias=par_ac[:, b + 3 : b + 4], scale=-140000.0,
                        )

                    # R-scaled units: pin value is exactly 1 = is_ge output, so
                    # d1~ = max((W>=thr), lam/R) is a single fused op.
                    d1_t = pool.tile([128, T], dt.bfloat16, tag="d1")
                    nc.gpsimd.tensor_scalar(
                        d1_t[:], w_t[:], 0.99975, par_gp[:, b + 4 : b + 5],
                        op.is_ge, op.max,
                    )
                    d0_t = pool.tile([128, T], dt.float16, tag="d0")
                    nc.gpsimd.tensor_tensor(d0_t[:], notm_t[:], a_t[:], op.mult)

                    p_t = pool.tile([128, T], dt.bfloat16, tag="p")
                    nc.vector.tensor_tensor_scan(
                        p_t[:], d0_t[:], d1_t[:], par_dv[:, b + 7 : b + 8],
                        op.mult, op.add,
                    )

                    nc.sync.dma_start(p_d[i * 128 : (i + 1) * 128, :], p_t[:])
    nc.compile()
    return nc


def _host_params(X, learn_w, guess_w, slip_w, prior_w):
    f32 = np.float32
    f64 = np.float64

    def sig(w):
        return 1.0 / (1.0 + np.exp(-w.astype(f64)))

    l = sig(learn_w[X[:, 0], 0])
    g = sig(guess_w[X[:, 1], 0])
    s = sig(slip_w[X[:, 2], 0])
    p = sig(prior_w[X[:, 3], 0])
    R = f64(_consts())
    a1 = (1 - s) / (g * (1 - l))
    a0 = s / ((1 - g) * (1 - l))
    lam = l / (1 - l)
    rho0 = p / (1 - p)
    d = (a1 - a0).astype(f32)
    w0 = (rho0 / R).astype(f32)
    zero = np.zeros_like(d)
    kb = np.full_like(d, 139965.0)        # sigmoid bias const (see _build_bass)
    lamR = (lam / R).astype(f32)
    par = np.stack(
        [a0.astype(f32), d, zero, kb, lamR,
         rho0.astype(f32), zero, w0], axis=1,
    )
    par = par.reshape(N_CORES, N_TILES, 128, NPAR).transpose(0, 2, 1, 3)
    par = np.ascontiguousarray(par.reshape(N_CORES, 128, N_TILES * NPAR), dtype=f32)
    gk = g.astype(f32)
    ck = (1 - s - g).astype(f32)
    return par, gk, ck, p.astype(f32)


def kernel(X, y, learn_w, guess_w, slip_w, prior_w, _trace=False):
    from concourse import bass_utils

    X = np.asarray(X)
    y8 = np.ascontiguousarray(np.asarray(y, dtype=np.int8))
    par, gk, ck, p0 = _host_params(
        X,
        np.asarray(learn_w, np.float32),
        np.asarray(guess_w, np.float32),
        np.asarray(slip_w, np.float32),
        np.asarray(prior_w, np.float32),
    )

    if "nc" not in _cache:
        _cache["nc"] = _build_bass()
    nc = _cache["nc"]

    in_maps = [
        {"y": y8[i * B_CORE : (i + 1) * B_CORE], "par": par[i]}
        for i in range(N_CORES)
    ]
    res = bass_utils.run_bass_kernel_spmd(
        nc, in_maps, core_ids=list(range(N_CORES)), trace=_trace
    )
    outs = res.results
    p_all = np.concatenate(
        [np.asarray(outs[i]["p"]).astype(np.float32) for i in range(N_CORES)], axis=0
    )
    # p_all[:, t] = odds/R AFTER step t; latents are recorded BEFORE the update
    lat = np.empty((B_FULL, T), np.float32)
    lat[:, 0] = p0
    ptrim = p_all[:, : T - 1] * np.float32(_consts())
    lat[:, 1:] = ptrim / (1.0 + ptrim)
    cor = gk[:, None] + ck[:, None] * lat
    if _trace:
        _cache["last_exec_time_ns"] = res.exec_time_ns
    return cor, lat


# revision 24
# speedup vs baseline: 1.0644x; 1.0351x over previous
"""BKT forward kernel for Trainium2 (8 NeuronCores, data-parallel over batch).

Math: in odds space rho = L/(1-L) the BKT update is affine:
    rho' = a_t * rho + lam,   a_t = y ? (1-s)/(g(1-l)) : s/((1-g)(1-l)),
and the clip L <= 1-EPS becomes rho <= R. Pin steps (clip binding) are
detected with a linear-space scaled scan W' = min(a*W, 1) (W = rho_mult/R,
fp32 scan state so no underflow for any reachable trajectory), threshold
W >= (R-lam)/R. The trajectory is then rebuilt with a mult/add scan whose
operands force state = R at pins.

Engine schedule (per 128-student tile, all [128,512]):
  Act : a    = y*d + a0                      -> fp16  (612ns)
  DVE : W    = scan(a*state min 1)           -> fp16  (594ns, fp32 state)
  Act : notm = sigmoid(W*(-K) + K*theta)     -> fp16  (612ns, saturated step)
  Pool: t1   = (W >= theta)*R                -> bf16  (~427ns)
  Pool: d1   = max(t1, lam)                  -> bf16  (~427ns)
  Pool: d0   = notm * a                      -> fp16  (~427ns)
  DVE : p    = scan(d0*state add d1)         -> bf16  (594ns, fp32 state)
  SP  : DMA y in, DMA p out (bf16)
Host computes lat = p/(1+p), cor = g+(1-s-g)*lat (bounded maps of p, so
bf16 output error stays ~0.4%).
"""

import numpy as np

B_FULL = 65536
T = 512
N_CORES = 8
B_CORE = B_FULL // N_CORES          # 8192
N_TILES = B_CORE // 128             # 64
EPS = 1e-6
NPAR = 8

_cache = {}


def _consts():
    f32 = np.float32
    Lstar = f32(1.0) - f32(EPS)
    R = f32(np.float64(Lstar) / (1.0 - np.float64(Lstar)))
    return float(R)


def _build_bass():
    import concourse.bacc as bacc
    import concourse.mybir as mybir
    from concourse.tile import TileContext

    R = _consts()
    dt = mybir.dt
    op = mybir.AluOpType
    act = mybir.ActivationFunctionType

    nc = bacc.Bacc(None, target_bir_lowering=False)
    y_d = nc.dram_tensor("y", [B_CORE, T], dt.int8, kind="ExternalInput")
    par_d = nc.dram_tensor("par", [128, N_TILES * NPAR], dt.float32, kind="ExternalInput")
    p_d = nc.dram_tensor("p", [B_CORE, T], dt.bfloat16, kind="ExternalOutput")

    # par slots: 0=a0, 1=d, 2=negKp, 3=Kb, 4=lam, 5=rho0, 6=theta, 7=w0
    with TileContext(nc) as tc:
        with (
            tc.tile_pool(name="const", bufs=1) as cpool,
            tc.tile_pool(name="work", bufs=8) as pool,
            tc.tile_pool(name="grp", bufs=3) as gpool,
        ):
            ones16 = cpool.tile([128, T], dt.float16)
            nc.gpsimd.memset(ones16[:], 1.0)
            par_t = cpool.tile([128, N_TILES * NPAR], dt.float32)
            # gpsimd-issued so the first y DMA (SP queue) runs in parallel
            nc.gpsimd.dma_start(par_t[:], par_d[:, :])
            # per-engine copies so scalar-AP reads are same-engine deps
            par_gp = cpool.tile([128, N_TILES * NPAR], dt.float32)
            nc.gpsimd.tensor_copy(par_gp[:], par_t[:])
            par_dv = cpool.tile([128, N_TILES * NPAR], dt.float32)
            nc.vector.tensor_copy(par_dv[:], par_t[:])
            par_ac = cpool.tile([128, N_TILES * NPAR], dt.float32)
            nc.scalar.copy(par_ac[:], par_t[:])

            # Tile groups: the p-scan is chainable across students via
            # separator columns (d0=0, d1=w0 re-initializes the state), so
            # 4 tiles share one 2052-wide scan, amortizing per-instruction
            # overhead. Tail groups stay small to keep the drain chain short.
            groups = [list(range(4 * g, 4 * g + 4)) for g in range(15)]
            groups += [[60, 61], [62], [63]]
            S = T + 1  # per-tile segment width in the batched scan

            # 2-stage software pipeline: stage A (dma/a/W) of group g+1 is
            # emitted before stage B (notm/d1/d0/p/dma) of group g so the
            # DVE runs W-scans while Act/Pool produce scan operands.
            stash = {}
            for gi in range(len(groups) + 1):
                if gi < len(groups):
                    for j in groups[gi]:
                        b = j * NPAR
                        y_t = pool.tile([128, T], dt.int8, tag="y")
                        nc.sync.dma_start(y_t[:], y_d[j * 128 : (j + 1) * 128, :])

                        a_t = pool.tile([128, T], dt.float16, tag="a")
                        if j % 7 < 4:
                            nc.scalar.activation(
                                a_t[:], y_t[:], act.Identity,
                                bias=par_ac[:, b + 0 : b + 1],
                                scale=par_ac[:, b + 1 : b + 2],
                            )
                        else:
                            nc.gpsimd.tensor_scalar(
                                a_t[:], y_t[:], par_gp[:, b + 1 : b + 2],
                                par_gp[:, b + 0 : b + 1], op.mult, op.add,
                            )

                        w_t = pool.tile([128, T], dt.float16, tag="w")
                        nc.vector.tensor_tensor_scan(
                            w_t[:], a_t[:], ones16[:], par_dv[:, b + 7 : b + 8],
                            op.mult, op.min,
                        )
                        stash[j] = (a_t, w_t)

                if gi >= 1:
                    G = groups[gi - 1]
                    n = len(G)
                    d0g = gpool.tile([128, S * n], dt.float16, tag=f"d0g{n}")
                    d1g = gpool.tile([128, S * n], dt.bfloat16, tag=f"d1g{n}")
                    # separator columns: d0=0, d1=w0 (state re-init per tile)
                    nc.gpsimd.memset(d0g[:, 0 : S * n : S], 0.0)
                    b0 = G[0] * NPAR
                    nc.gpsimd.tensor_copy(
                        d1g[:, 0 : S * n : S],
                        par_gp[:, b0 + 7 : b0 + 7 + (n - 1) * NPAR + 1 : NPAR],
                    )
                    for k, i in enumerate(G):
                        b = i * NPAR
                        a_t, w_t = stash.pop(i)
                        # fp16 W is quantized: no values in (1-4.88e-4, 1), so
                        # global threshold 0.99975 separates pinned (W==1)
                        # from unpinned. sigmoid arg = -140000*W + 139965:
                        # exactly -35 at W=1 (notm->0), +33.4 one ulp below.
                        notm_t = pool.tile([128, T], dt.float16, tag="notm")
                        nc.scalar.activation(
                            notm_t[:], w_t[:], act.Sigmoid,
                            bias=par_ac[:, b + 3 : b + 4], scale=-140000.0,
                        )
                        # R-scaled units: pin value is exactly 1 = is_ge
                        # output, so d1~ = max((W>=thr), lam/R) is one op.
                        nc.gpsimd.tensor_scalar(
                            d1g[:, S * k + 1 : S * k + S], w_t[:], 0.99975,
                            par_gp[:, b + 4 : b + 5], op.is_ge, op.max,
                        )
                        nc.gpsimd.tensor_tensor(
                            d0g[:, S * k + 1 : S * k + S], notm_t[:], a_t[:],
                            op.mult,
                        )

                    p_g = gpool.tile([128, S * n], dt.bfloat16, tag=f"pg{n}")
                    nc.vector.tensor_tensor_scan(
                        p_g[:], d0g[:], d1g[:], 0.0, op.mult, op.add,
                    )
                    for k, i in enumerate(G):
                        nc.sync.dma_start(
                            p_d[i * 128 : (i + 1) * 128, :],
                            p_g[:, S * k : S * k + T],
                        )
    nc.compile()
    return nc


def _host_params(X, learn_w, guess_w, slip_w, prior_w):
    f32 = np.float32
    f64 = np.float64

    def sig(w):
        return 1.0 / (1.0 + np.exp(-w.astype(f64)))

    l = sig(learn_w[X[:, 0], 0])
    g = sig(guess_w[X[:, 1], 0])
    s = sig(slip_w[X[:, 2], 0])
    p = sig(prior_w[X[:, 3], 0])
    R = f64(_consts())
    a1 = (1 - s) / (g * (1 - l))
    a0 = s / ((1 - g) * (1 - l))
    lam = l / (1 - l)
    rho0 = p / (1 - p)
    d = (a1 - a0).astype(f32)
    w0 = (rho0 / R).astype(f32)
    zero = np.zeros_like(d)
    kb = np.full_like(d, 139965.0)        # sigmoid bias const (see _build_bass)
    lamR = (lam / R).astype(f32)
    par = np.stack(
        [a0.astype(f32), d, zero, kb, lamR,
         rho0.astype(f32), zero, w0], axis=1,
    )
    par = par.reshape(N_CORES, N_TILES, 128, NPAR).transpose(0, 2, 1, 3)
    par = np.ascontiguousarray(par.reshape(N_CORES, 128, N_TILES * NPAR), dtype=f32)
    gk = g.astype(f32)
    ck = (1 - s - g).astype(f32)
    return par, gk, ck, p.astype(f32)


def kernel(X, y, learn_w, guess_w, slip_w, prior_w, _trace=False):
    from concourse import bass_utils

    X = np.asarray(X)
    y8 = np.ascontiguousarray(np.asarray(y, dtype=np.int8))
    par, gk, ck, p0 = _host_params(
        X,
        np.asarray(learn_w, np.float32),
        np.asarray(guess_w, np.float32),
        np.asarray(slip_w, np.float32),
        np.asarray(prior_w, np.float32),
    )

    if "nc" not in _cache:
        _cache["nc"] = _build_bass()
    nc = _cache["nc"]

    in_maps = [
        {"y": y8[i * B_CORE : (i + 1) * B_CORE], "par": par[i]}
        for i in range(N_CORES)
    ]
    res = bass_utils.run_bass_kernel_spmd(
        nc, in_maps, core_ids=list(range(N_CORES)), trace=_trace
    )
    outs = res.results
    p_all = np.concatenate(
        [np.asarray(outs[i]["p"]).astype(np.float32) for i in range(N_CORES)], axis=0
    )
    # batched-scan layout: col t of each tile segment = odds/R BEFORE step t
    rp = p_all * np.float32(_consts())
    lat = rp / (1.0 + rp)
    lat[:, 0] = p0
    cor = gk[:, None] + ck[:, None] * lat
    if _trace:
        _cache["last_exec_time_ns"] = res.exec_time_ns
    return cor, lat
